# revision 13
# baseline (speedup 1.0000x reference)
"""Causal self-attention on 8 Trainium2 NeuronCores (Bass/Tile).

Problem: nn_CausalSelfAttention (B=4, T=2048, C=1024, H=16 heads, fp32).

Sharding: tensor-parallel over heads for QKV projection + attention
(2 heads per core), AllGather of per-core attention outputs (transposed
layout, 4 MB/core), then tensor-parallel over output columns for the
final projection (each core computes a 128-column slice of x@W_proj).

Layouts (everything "transposed", i.e. feature dim on partitions):
  xT      [C, B*T]        input, replicated to all cores
  Q^T,K^T [CH, B*T]       CH = 128 channels (2 heads x 64) per core
  V       [B*T, CH]       natural layout (needed as matmul lhsT for P@V),
                          stored tiled with an extra ones-column per head so
                          the PV matmul also produces softmax denominators
  S^T     [kr, q] tiles   scores transposed: kr on partitions -> the exp'd
                          tiles feed P@V directly as the moving operand
  attn^T  [CH, B*T]       per-core attention out -> AllGather -> [C, B*T]
  y^T     [OC, B*T]       per-core 128-column slice of the final output

Softmax is unnormalized exp (no max subtraction; scores are O(1) by
construction) with the causal mask applied as an accumulated -1e5 constant
via an identity matmul on diagonal tiles; the denominator is produced by the
ones-column in the PV matmul and divided out with a PE-broadcast reciprocal.
All matmuls run as float32r (single-pass fp32 on the PE array).
"""

import numpy as np
from contextlib import ExitStack

P = 128
NQ = 512  # q/moving-operand tile width
MASKVAL = -1.0e5


def build_attention_nc(B, T, C, H, n_cores):
    import concourse.bass as bass  # noqa: F401
    import concourse.bacc as bacc
    import concourse.tile as tile
    import concourse.mybir as mybir

    f32 = mybir.dt.float32
    f32r = mybir.dt.float32r
    Identity = mybir.ActivationFunctionType.Identity
    Exp = mybir.ActivationFunctionType.Exp

    hs = C // H              # head size
    hpc = H // n_cores       # heads per core
    CH = hpc * hs            # qkv channels per core
    OC = C // n_cores        # output columns per core
    NT = B * T               # tokens
    KT_E = C // P            # contraction tiles over embedding dim
    NROW = NT // NQ          # token row-tiles
    TQ = T // NQ             # q tiles per batch
    TK = T // P              # kr tiles per batch
    TKALL = NT // P          # kr tiles over all batches
    DPB = NQ // P            # kr-tiles crossing one q-tile's diagonal
    WV = hpc * (hs + 1)      # V storage width per kr-tile (with ones cols)

    assert T % NQ == 0 and C % P == 0 and NT % NQ == 0
    assert CH <= P and H % n_cores == 0
    assert hs * hpc == CH and hpc in (1, 2)
    scale = 1.0 / float(np.sqrt(hs))

    nc = bacc.Bacc("TRN2", target_bir_lowering=False, debug=False,
                   num_devices=n_cores)

    xT = nc.dram_tensor("xT", [C, NT], f32r, kind="ExternalInput")
    wqkv = nc.dram_tensor("wqkv", [C, 3 * CH], f32r, kind="ExternalInput")
    bqkv = nc.dram_tensor("bqkv", [CH, 3], f32, kind="ExternalInput")
    wp = nc.dram_tensor("wp", [C, OC], f32r, kind="ExternalInput")
    bp = nc.dram_tensor("bp", [OC, 1], f32, kind="ExternalInput")
    yT = nc.dram_tensor("yT", [OC, NT], f32, kind="ExternalOutput")

    # constants baked into the NEFF
    ident_np = np.eye(P, dtype=np.float32)
    masks_np = np.zeros((P, DPB * NQ), dtype=np.float32)
    for j in range(DPB):
        for p_ in range(P):
            # keep (mask=0) iff q >= kr, i.e. col >= j*P + p
            masks_np[p_, j * NQ:(j + 1) * NQ][: min(j * P + p_, NQ)] = MASKVAL
    ident_dram = nc.inline_tensor(ident_np, name="ident_const")
    masks_dram = nc.inline_tensor(masks_np, name="masks_const")
    ones_dram = nc.inline_tensor(np.ones((P, hs), dtype=np.float32),
                                 name="ones_const")
    vones_dram = nc.inline_tensor(np.ones((P, TKALL * hpc), dtype=np.float32),
                                  name="vones_const")

    with tile.TileContext(nc) as tc, ExitStack() as ctx:
        const = ctx.enter_context(tc.tile_pool(name="const", bufs=1))
        big = ctx.enter_context(tc.tile_pool(name="big", bufs=1))
        xin = ctx.enter_context(tc.tile_pool(name="xin", bufs=2))
        evac = ctx.enter_context(tc.tile_pool(name="evac", bufs=3))
        pexp = ctx.enter_context(tc.tile_pool(name="pexp", bufs=3))
        psA = ctx.enter_context(tc.tile_pool(name="psA", bufs=2, space="PSUM"))
        stp = ctx.enter_context(tc.tile_pool(name="stp", bufs=4, space="PSUM"))
        pvp = ctx.enter_context(tc.tile_pool(name="pvp", bufs=2, space="PSUM"))
        dram = ctx.enter_context(tc.tile_pool(name="dram", bufs=1, space="DRAM"))

        ident_sb = const.tile([P, P], f32r)
        ident_t = const.tile([P, P], f32)
        masks_sb = const.tile([P, DPB * NQ], f32r)
        ones_sb = const.tile([P, hs], f32r)
        bqkv_sb = const.tile([CH, 3], f32)
        bp_sb = const.tile([OC, 1], f32)
        w_sb = const.tile([P, KT_E * 3 * CH], f32r)
        wp_sb = const.tile([P, KT_E * OC], f32r)

        nc.sync.dma_start(ident_sb[:], ident_dram[:].bitcast(f32r))
        nc.sync.dma_start(ident_t[:], ident_dram[:])
        nc.sync.dma_start(masks_sb[:], masks_dram[:].bitcast(f32r))
        nc.sync.dma_start(ones_sb[:], ones_dram[:].bitcast(f32r))
        nc.sync.dma_start(bqkv_sb[:], bqkv[:])
        nc.sync.dma_start(bp_sb[:], bp[:])
        nc.sync.dma_start(
            w_sb[:].rearrange("p (k m) -> p k m", k=KT_E),
            wqkv[:].rearrange("(k p) m -> p k m", p=P),
        )
        nc.sync.dma_start(
            wp_sb[:].rearrange("p (k m) -> p k m", k=KT_E),
            wp[:].rearrange("(k p) m -> p k m", p=P),
        )

        QT = big.tile([CH, NT], f32r)
        KTt = big.tile([CH, NT], f32r)
        Vn = big.tile([P, TKALL * WV], f32r)

        # ones columns of V (softmax denominator trick)
        ones_view = Vn[:].rearrange("p (v h d) -> p v h d", h=hpc, d=hs + 1)[
            :, :, :, hs:hs + 1
        ]
        nc.sync.dma_start(
            ones_view,
            vones_dram[:].rearrange("p (v h d) -> p v h d", h=hpc, d=1)
            .bitcast(mybir.dt.float32r),
        )

        # ---- Phase A: QKV projection (outputs transposed) ----
        for n in range(NROW):
            ns = n * NQ
            xt = xin.tile([P, KT_E * NQ], f32r, tag="xcol")
            nc.sync.dma_start(
                xt[:].rearrange("p (k q) -> p k q", k=KT_E),
                xT[:, ns:ns + NQ].rearrange("(k p) q -> p k q", p=P),
            )
            for m in range(3):
                ps = psA.tile([P, NQ], f32, tag="mm")
                for k in range(KT_E):
                    nc.tensor.matmul(
                        ps[0:CH, :],
                        lhsT=w_sb[:, k * 3 * CH + m * CH:k * 3 * CH + (m + 1) * CH],
                        rhs=xt[:, k * NQ:(k + 1) * NQ],
                        start=(k == 0),
                        stop=(k == KT_E - 1),
                    )
                if m == 0:
                    nc.scalar.activation(QT[:, ns:ns + NQ], ps[0:CH, :],
                                         Identity, bias=bqkv_sb[:, 0:1])
                elif m == 1:
                    nc.scalar.activation(KTt[:, ns:ns + NQ], ps[0:CH, :],
                                         Identity, bias=bqkv_sb[:, 1:2])
                else:
                    vt = evac.tile([CH, NQ], f32, tag="vtmp")
                    nc.scalar.activation(vt[:], ps[0:CH, :],
                                         Identity, bias=bqkv_sb[:, 2:3])
                    # transpose the 4 [CH, P] blocks -> [P, CH] into one bank
                    tp = psA.tile([P, NQ], f32, tag="mm")
                    for j in range(DPB):
                        nc.tensor.transpose(
                            tp[:, j * CH:(j + 1) * CH],
                            vt[:, j * P:(j + 1) * P],
                            ident_t[0:CH, 0:CH],
                        )
                    # scatter into V storage (skipping the ones columns)
                    vi0 = n * DPB
                    dst = Vn[:, vi0 * WV:(vi0 + DPB) * WV].rearrange(
                        "p (v h d) -> p v h d", h=hpc, d=hs + 1
                    )[:, :, :, 0:hs]
                    src = tp[:, 0:DPB * CH].rearrange(
                        "p (v h d) -> p v h d", h=hpc, d=hs)
                    nc.vector.tensor_copy(dst, src)

        # ---- Phase B: attention per (batch, head) ----
        cc_in = dram.tile([CH, NT], f32)
        for b in range(B):
            for qt in range(TQ):
                qs = b * T + qt * NQ
                nkt = DPB * qt + DPB
                pvs = [pvp.tile([P, NQ], f32, tag="pv", name=f"pv{_h}")
                       for _h in range(hpc)]
                for kt in range(nkt):
                    ks = b * T + kt * P
                    vi = b * TK + kt
                    diag = kt >= DPB * qt
                    j = kt - DPB * qt
                    for hh in range(hpc):
                        st = stp.tile([P, NQ], f32, tag="st")
                        nc.tensor.matmul(
                            st[:],
                            lhsT=KTt[hh * hs:(hh + 1) * hs, ks:ks + P],
                            rhs=QT[hh * hs:(hh + 1) * hs, qs:qs + NQ],
                            start=True,
                            stop=not diag,
                        )
                        if diag:
                            nc.tensor.matmul(
                                st[:],
                                lhsT=ident_sb[:],
                                rhs=masks_sb[:, j * NQ:(j + 1) * NQ],
                                start=False,
                                stop=True,
                            )
                        pe = pexp.tile([P, NQ], f32r, tag="pe")
                        nc.scalar.activation(pe[:], st[:], Exp, scale=scale)
                        nc.tensor.matmul(
                            pvs[hh][0:hs + 1, :],
                            lhsT=Vn[:, vi * WV + hh * (hs + 1):
                                    vi * WV + (hh + 1) * (hs + 1)],
                            rhs=pe[:],
                            start=(kt == 0),
                            stop=(kt == nkt - 1),
                        )
                for hh in range(hpc):
                    rec = evac.tile([P, NQ], f32r, tag="rec")
                    with nc.allow_low_precision(reason="f32r recip for PE bcast"):
                        nc.vector.reciprocal(rec[hs:hs + 1, :],
                                             pvs[hh][hs:hs + 1, :])
                    bc = stp.tile([P, NQ], f32, tag="st")
                    nc.tensor.matmul(
                        bc[0:hs, :],
                        lhsT=ones_sb[hs:hs + 1, :],
                        rhs=rec[hs:hs + 1, :],
                        start=True,
                        stop=True,
                    )
                    bcs = evac.tile([hs, NQ], f32, tag="bcs")
                    nc.vector.tensor_copy(bcs[:], bc[0:hs, :])
                    ao = evac.tile([hs, NQ], f32, tag="ao")
                    nc.vector.tensor_mul(ao[:], pvs[hh][0:hs, :], bcs[:])
                    nc.sync.dma_start(
                        cc_in[hh * hs:(hh + 1) * hs, qs:qs + NQ], ao[:]
                    )

        # ---- Phase C: AllGather + output projection ----
        cc_out = dram.tile([n_cores * CH, NT], f32, addr_space="Shared")
        nc.gpsimd.collective_compute(
            "AllGather",
            mybir.AluOpType.bypass,
            replica_groups=[list(range(n_cores))],
            ins=[cc_in[:].opt()],
            outs=[cc_out[:].opt()],
        )
        for n in range(NROW):
            ns = n * NQ
            rt = xin.tile([P, KT_E * NQ], f32r, tag="xcol")
            nc.sync.dma_start(
                rt[:].rearrange("p (k q) -> p k q", k=KT_E),
                cc_out[:, ns:ns + NQ].rearrange("(k p) q -> p k q", p=P).bitcast(f32r),
            )
            ps = psA.tile([P, NQ], f32, tag="mm")
            for k in range(KT_E):
                nc.tensor.matmul(
                    ps[0:OC, :],
                    lhsT=wp_sb[:, k * OC:(k + 1) * OC],
                    rhs=rt[:, k * NQ:(k + 1) * NQ],
                    start=(k == 0),
                    stop=(k == KT_E - 1),
                )
            yo = evac.tile([OC, NQ], f32, tag="yo")
            nc.scalar.activation(yo[:], ps[0:OC, :], Identity, bias=bp_sb[:, 0:1])
            nc.sync.dma_start(yT[:, ns:ns + NQ], yo[:])

    nc.compile()
    return nc


def shard_inputs(x, W_qkv, b_qkv, W_proj, b_proj, H, n_cores):
    B, T, C = x.shape
    hs = C // H
    hpc = H // n_cores
    CH = hpc * hs
    OC = C // n_cores
    x2 = np.asarray(x, dtype=np.float32).reshape(B * T, C)
    xT = np.ascontiguousarray(x2.T)
    W_qkv = np.asarray(W_qkv, dtype=np.float32)
    b_qkv = np.asarray(b_qkv, dtype=np.float32)
    W_proj = np.asarray(W_proj, dtype=np.float32)
    b_proj = np.asarray(b_proj, dtype=np.float32)
    in_maps = []
    for i in range(n_cores):
        sl = slice(i * CH, (i + 1) * CH)
        wqkv_i = np.ascontiguousarray(np.concatenate(
            [W_qkv[:, sl], W_qkv[:, C:][:, sl], W_qkv[:, 2 * C:][:, sl]], axis=1))
        bqkv_i = np.ascontiguousarray(np.stack(
            [b_qkv[sl], b_qkv[C:][sl], b_qkv[2 * C:][sl]], axis=1))
        wp_i = np.ascontiguousarray(W_proj[:, i * OC:(i + 1) * OC])
        bp_i = np.ascontiguousarray(b_proj[i * OC:(i + 1) * OC].reshape(OC, 1))
        in_maps.append({"xT": xT, "wqkv": wqkv_i, "bqkv": bqkv_i,
                        "wp": wp_i, "bp": bp_i})
    return in_maps


def gather_output(results, B, T, C, n_cores):
    yT = np.concatenate([results[i]["yT"] for i in range(n_cores)], axis=0)
    return np.ascontiguousarray(yT.T).reshape(B, T, C).astype(np.float32)


_NC_CACHE = {}


def _get_nc(B, T, C, H, n_cores):
    key = (B, T, C, H, n_cores)
    if key not in _NC_CACHE:
        _NC_CACHE[key] = build_attention_nc(B, T, C, H, n_cores)
    return _NC_CACHE[key]


def kernel(x, W_qkv, b_qkv, W_proj, b_proj):
    from concourse import bass_utils

    B, T, C = 4, 2048, 1024
    H, n_cores = 16, 8
    assert x.shape == (B, T, C)
    nc = _get_nc(B, T, C, H, n_cores)
    in_maps = shard_inputs(x, W_qkv, b_qkv, W_proj, b_proj, H, n_cores)
    res = bass_utils.run_bass_kernel_spmd(
        nc, in_maps, core_ids=list(range(n_cores))
    )
    return gather_output(res.results, B, T, C, n_cores)


# revision 16
# speedup vs baseline: 1.5107x; 1.5107x over previous
"""Causal self-attention on 8 Trainium2 NeuronCores (Bass/Tile).

Problem: nn_CausalSelfAttention (B=4, T=2048, C=1024, H=16 heads, fp32).

Sharding: tensor-parallel over heads for QKV projection + attention
(2 heads per core), per-batch AllGather of attention outputs (fp16,
transposed layout), then tensor-parallel over output columns for the
final projection (each core computes a 128-column slice of x@W_proj).
The AllGather/projection of batch b runs concurrently with the
attention of batch b+1.

Layouts (feature dim on partitions everywhere):
  xT      [C, B*T]        input, replicated to all cores
  Q^T,K^T [CH, B*T]       CH = channels per core (2 heads x 64)
  V       [B*T, CH]       natural layout (matmul lhsT for P@V), stored
                          tiled with an extra ones-column per head so the
                          PV matmul also produces softmax denominators
  S^T     [kr, q] tiles   scores transposed: kr on partitions -> exp'd
                          tiles feed P@V directly as the moving operand
  attn^T  [CH, T] fp16    per-core, per-batch -> AllGather -> [C, T]
  y^T     [OC, B*T]       per-core 128-column slice of the final output

Softmax: unnormalized exp (scores are O(1), no max subtraction needed);
causal mask = DVE add of a -1e5 upper-triangular [128,128] constant onto
the single diagonal-crossing strip of each clipped tile; denominator from
the V ones-column; division via reciprocal_approx_fast + PE broadcast.
Matmuls run as float32r (single-pass fp32); proj runs fp16 inputs.
"""

import numpy as np
from contextlib import ExitStack

P = 128
NQ = 512  # q/moving-operand tile width
MASKVAL = -1.0e5


def build_attention_nc(B, T, C, H, n_cores):
    import concourse.bass as bass  # noqa: F401
    import concourse.bacc as bacc
    import concourse.tile as tile
    import concourse.mybir as mybir

    f32 = mybir.dt.float32
    f32r = mybir.dt.float32r
    fp16 = mybir.dt.float16
    Identity = mybir.ActivationFunctionType.Identity
    Exp = mybir.ActivationFunctionType.Exp

    hs = C // H              # head size
    hpc = H // n_cores       # heads per core
    CH = hpc * hs            # qkv channels per core
    OC = C // n_cores        # output columns per core
    NT = B * T               # tokens
    KT_E = C // P            # contraction tiles over embedding dim
    NROW = NT // NQ          # token row-tiles
    TQ = T // NQ             # q tiles per batch
    TK = T // P              # kr tiles per batch
    TKALL = NT // P          # kr tiles over all batches
    DPB = NQ // P            # kr-tiles crossing one q-tile's diagonal
    WV = hpc * (hs + 1)      # V storage width per kr-tile (with ones cols)

    assert T % NQ == 0 and C % P == 0 and NT % NQ == 0
    assert CH <= P and H % n_cores == 0
    assert hs * hpc == CH and hpc in (1, 2)
    scale = 1.0 / float(np.sqrt(hs))

    nc = bacc.Bacc("TRN2", target_bir_lowering=False, debug=False,
                   num_devices=n_cores)

    xT = nc.dram_tensor("xT", [C, NT], f32r, kind="ExternalInput")
    wqkv = nc.dram_tensor("wqkv", [C, 3 * CH], f32r, kind="ExternalInput")
    bqkv = nc.dram_tensor("bqkv", [CH, 3], f32, kind="ExternalInput")
    wp = nc.dram_tensor("wp", [C, OC], fp16, kind="ExternalInput")
    bp = nc.dram_tensor("bp", [OC, 1], f32, kind="ExternalInput")
    yT = nc.dram_tensor("yT", [OC, NT], f32, kind="ExternalOutput")

    # constants baked into the NEFF
    ident_np = np.eye(P, dtype=np.float32)
    # mask[p, c] = MASKVAL where kr-offset p > q-offset c (strictly lower)
    mask_np = np.ascontiguousarray(np.where(
        np.arange(P)[:, None] > np.arange(P)[None, :], MASKVAL, 0.0
    ).astype(np.float32))
    ident_dram = nc.inline_tensor(ident_np, name="ident_const")
    mask_dram = nc.inline_tensor(mask_np, name="mask_const")
    ones_dram = nc.inline_tensor(np.ones((P, hs), dtype=np.float32),
                                 name="ones_const")
    vones_dram = nc.inline_tensor(np.ones((P, TKALL * hpc), dtype=np.float32),
                                  name="vones_const")

    with tile.TileContext(nc) as tc, ExitStack() as ctx:
        const = ctx.enter_context(tc.tile_pool(name="const", bufs=1))
        big = ctx.enter_context(tc.tile_pool(name="big", bufs=1))
        xin = ctx.enter_context(tc.tile_pool(name="xin", bufs=2))
        evac = ctx.enter_context(tc.tile_pool(name="evac", bufs=3))
        pexp = ctx.enter_context(tc.tile_pool(name="pexp", bufs=6))
        psA = ctx.enter_context(tc.tile_pool(name="psA", bufs=2, space="PSUM"))
        stp = ctx.enter_context(tc.tile_pool(name="stp", bufs=4, space="PSUM"))
        pvp = ctx.enter_context(tc.tile_pool(name="pvp", bufs=2, space="PSUM"))
        dram = ctx.enter_context(tc.tile_pool(name="dram", bufs=1, space="DRAM"))

        ident_t = const.tile([P, P], f32)
        mask_sb = const.tile([P, P], f32)
        ones_sb = const.tile([P, hs], f32r)
        bqkv_sb = const.tile([CH, 3], f32)
        bp_sb = const.tile([OC, 1], f32)
        w_sb = const.tile([P, KT_E * 3 * CH], f32r)
        wp_sb = const.tile([P, KT_E * OC], fp16)

        nc.sync.dma_start(ident_t[:], ident_dram[:])
        nc.sync.dma_start(mask_sb[:], mask_dram[:])
        nc.sync.dma_start(ones_sb[:], ones_dram[:].bitcast(f32r))
        nc.sync.dma_start(bqkv_sb[:], bqkv[:])
        nc.sync.dma_start(bp_sb[:], bp[:])
        nc.sync.dma_start(
            w_sb[:].rearrange("p (k m) -> p k m", k=KT_E),
            wqkv[:].rearrange("(k p) m -> p k m", p=P),
        )
        nc.sync.dma_start(
            wp_sb[:].rearrange("p (k m) -> p k m", k=KT_E),
            wp[:].rearrange("(k p) m -> p k m", p=P),
        )

        QT = big.tile([CH, NT], f32r)
        KTt = big.tile([CH, NT], f32r)
        Vn = big.tile([P, TKALL * WV], f32r)

        # ones columns of V (softmax denominator trick)
        ones_view = Vn[:].rearrange("p (v h d) -> p v h d", h=hpc, d=hs + 1)[
            :, :, :, hs:hs + 1
        ]
        nc.sync.dma_start(
            ones_view,
            vones_dram[:].rearrange("p (v h d) -> p v h d", h=hpc, d=1)
            .bitcast(f32r),
        )

        # ---- Phase A: QKV projection (outputs transposed) ----
        for n in range(NROW):
            ns = n * NQ
            xt = xin.tile([P, KT_E * NQ], f32r, tag="xcol")
            nc.sync.dma_start(
                xt[:].rearrange("p (k q) -> p k q", k=KT_E),
                xT[:, ns:ns + NQ].rearrange("(k p) q -> p k q", p=P),
            )
            for m in range(3):
                ps = psA.tile([P, NQ], f32, tag="mm")
                for k in range(KT_E):
                    nc.tensor.matmul(
                        ps[0:CH, :],
                        lhsT=w_sb[:, k * 3 * CH + m * CH:k * 3 * CH + (m + 1) * CH],
                        rhs=xt[:, k * NQ:(k + 1) * NQ],
                        start=(k == 0),
                        stop=(k == KT_E - 1),
                    )
                if m == 0:
                    nc.scalar.activation(QT[:, ns:ns + NQ], ps[0:CH, :],
                                         Identity, bias=bqkv_sb[:, 0:1])
                elif m == 1:
                    nc.scalar.activation(KTt[:, ns:ns + NQ], ps[0:CH, :],
                                         Identity, bias=bqkv_sb[:, 1:2])
                else:
                    vt = evac.tile([CH, NQ], f32, tag="vtmp")
                    nc.scalar.activation(vt[:], ps[0:CH, :],
                                         Identity, bias=bqkv_sb[:, 2:3])
                    # transpose the DPB [CH, P] blocks -> [P, CH] into one bank
                    tp = psA.tile([P, NQ], f32, tag="mm")
                    for j in range(DPB):
                        nc.tensor.transpose(
                            tp[:, j * CH:(j + 1) * CH],
                            vt[:, j * P:(j + 1) * P],
                            ident_t[0:CH, 0:CH],
                        )
                    # scatter into V storage (skipping the ones columns)
                    vi0 = n * DPB
                    dst = Vn[:, vi0 * WV:(vi0 + DPB) * WV].rearrange(
                        "p (v h d) -> p v h d", h=hpc, d=hs + 1
                    )[:, :, :, 0:hs]
                    src = tp[:, 0:DPB * CH].rearrange(
                        "p (v h d) -> p v h d", h=hpc, d=hs)
                    nc.vector.tensor_copy(dst, src)

        # ---- Phase B + C interleaved per batch ----
        cc_ins = [dram.tile([CH, T], fp16, name=f"ccin{b}") for b in range(B)]
        cc_outs = [dram.tile([n_cores * CH, T], fp16, addr_space="Shared",
                             name=f"ccout{b}") for b in range(B)]

        def attention(b):
            for qt in range(TQ):
                qs = b * T + qt * NQ
                nkt = DPB * qt + DPB
                pvs = [pvp.tile([P, NQ], f32, tag="pv", name=f"pv{_h}")
                       for _h in range(hpc)]
                pes = {}

                def issue_st(kt, qt=qt, qs=qs, pes=pes):
                    ks = b * T + kt * P
                    diag = kt >= DPB * qt
                    j = kt - DPB * qt
                    c0 = j * P if diag else 0
                    for hh in range(hpc):
                        st = stp.tile([P, NQ], f32, tag="st", name=f"st{hh}")
                        nc.tensor.matmul(
                            st[:, c0:NQ],
                            lhsT=KTt[hh * hs:(hh + 1) * hs, ks:ks + P],
                            rhs=QT[hh * hs:(hh + 1) * hs, qs + c0:qs + NQ],
                            start=True,
                            stop=True,
                        )
                        if diag:
                            nc.vector.tensor_add(
                                st[:, c0:c0 + P], st[:, c0:c0 + P], mask_sb[:])
                        pe_t = pexp.tile([P, NQ], f32r, tag="pe",
                                         name=f"pe{hh}")
                        nc.scalar.activation(pe_t[:, c0:NQ], st[:, c0:NQ],
                                             Exp, scale=scale)
                        pes[(kt, hh)] = (pe_t, c0)

                def issue_pv(kt, nkt=nkt, pvs=pvs, pes=pes):
                    vi = b * TK + kt
                    for hh in range(hpc):
                        pe_t, c0 = pes.pop((kt, hh))
                        nc.tensor.matmul(
                            pvs[hh][0:hs + 1, c0:NQ],
                            lhsT=Vn[:, vi * WV + hh * (hs + 1):
                                    vi * WV + (hh + 1) * (hs + 1)],
                            rhs=pe_t[:, c0:NQ],
                            start=(kt == 0),
                            stop=(kt == nkt - 1),
                        )

                for kt in range(nkt):
                    issue_st(kt)
                    if kt >= 2:
                        issue_pv(kt - 2)
                for kt in range(max(0, nkt - 2), nkt):
                    issue_pv(kt)

                for hh in range(hpc):
                    rec = evac.tile([P, NQ], f32r, tag="rec")
                    with nc.allow_low_precision(reason="recip for bcast"):
                        nc.vector.reciprocal(rec[hs:hs + 1, :],
                                             pvs[hh][hs:hs + 1, :])
                    bc = stp.tile([P, NQ], f32, tag="st")
                    nc.tensor.matmul(
                        bc[0:hs, :],
                        lhsT=ones_sb[hs:hs + 1, :],
                        rhs=rec[hs:hs + 1, :],
                        start=True,
                        stop=True,
                    )
                    bcs = evac.tile([hs, NQ], f32, tag="bcs")
                    nc.vector.tensor_copy(bcs[:], bc[0:hs, :])
                    ao = evac.tile([hs, NQ], fp16, tag="ao")
                    nc.vector.tensor_mul(ao[:], pvs[hh][0:hs, :], bcs[:])
                    nc.sync.dma_start(
                        cc_ins[b][hh * hs:(hh + 1) * hs,
                                  qt * NQ:(qt + 1) * NQ],
                        ao[:],
                    )

        def proj(b):
            for n in range(T // NQ):
                ns = n * NQ
                rt = xin.tile([P, KT_E * NQ], fp16, tag="xcol")
                nc.sync.dma_start(
                    rt[:].rearrange("p (k q) -> p k q", k=KT_E),
                    cc_outs[b][:, ns:ns + NQ].rearrange("(k p) q -> p k q",
                                                        p=P),
                )
                ps = psA.tile([P, NQ], f32, tag="mm")
                for k in range(KT_E):
                    nc.tensor.matmul(
                        ps[0:OC, :],
                        lhsT=wp_sb[:, k * OC:(k + 1) * OC],
                        rhs=rt[:, k * NQ:(k + 1) * NQ],
                        start=(k == 0),
                        stop=(k == KT_E - 1),
                    )
                yo = evac.tile([OC, NQ], f32, tag="yo")
                nc.scalar.activation(yo[:], ps[0:OC, :], Identity,
                                     bias=bp_sb[:, 0:1])
                nc.sync.dma_start(yT[:, b * T + ns:b * T + ns + NQ], yo[:])

        for b in range(B):
            attention(b)
            nc.gpsimd.collective_compute(
                "AllGather",
                mybir.AluOpType.bypass,
                replica_groups=[list(range(n_cores))],
                ins=[cc_ins[b][:].opt()],
                outs=[cc_outs[b][:].opt()],
            )
            if b >= 1:
                proj(b - 1)
        proj(B - 1)

    nc.compile()
    return nc


def shard_inputs(x, W_qkv, b_qkv, W_proj, b_proj, H, n_cores):
    B, T, C = x.shape
    hs = C // H
    hpc = H // n_cores
    CH = hpc * hs
    OC = C // n_cores
    x2 = np.asarray(x, dtype=np.float32).reshape(B * T, C)
    xT = np.ascontiguousarray(x2.T)
    W_qkv = np.asarray(W_qkv, dtype=np.float32)
    b_qkv = np.asarray(b_qkv, dtype=np.float32)
    W_proj = np.asarray(W_proj, dtype=np.float32)
    b_proj = np.asarray(b_proj, dtype=np.float32)
    in_maps = []
    for i in range(n_cores):
        sl = slice(i * CH, (i + 1) * CH)
        wqkv_i = np.ascontiguousarray(np.concatenate(
            [W_qkv[:, sl], W_qkv[:, C:][:, sl], W_qkv[:, 2 * C:][:, sl]], axis=1))
        bqkv_i = np.ascontiguousarray(np.stack(
            [b_qkv[sl], b_qkv[C:][sl], b_qkv[2 * C:][sl]], axis=1))
        wp_i = np.ascontiguousarray(
            W_proj[:, i * OC:(i + 1) * OC].astype(np.float16))
        bp_i = np.ascontiguousarray(b_proj[i * OC:(i + 1) * OC].reshape(OC, 1))
        in_maps.append({"xT": xT, "wqkv": wqkv_i, "bqkv": bqkv_i,
                        "wp": wp_i, "bp": bp_i})
    return in_maps


def gather_output(results, B, T, C, n_cores):
    yT = np.concatenate([results[i]["yT"] for i in range(n_cores)], axis=0)
    return np.ascontiguousarray(yT.T).reshape(B, T, C).astype(np.float32)


_NC_CACHE = {}


def _get_nc(B, T, C, H, n_cores):
    key = (B, T, C, H, n_cores)
    if key not in _NC_CACHE:
        _NC_CACHE[key] = build_attention_nc(B, T, C, H, n_cores)
    return _NC_CACHE[key]


def kernel(x, W_qkv, b_qkv, W_proj, b_proj):
    from concourse import bass_utils

    B, T, C = 4, 2048, 1024
    H, n_cores = 16, 8
    assert x.shape == (B, T, C)
    nc = _get_nc(B, T, C, H, n_cores)
    in_maps = shard_inputs(x, W_qkv, b_qkv, W_proj, b_proj, H, n_cores)
    res = bass_utils.run_bass_kernel_spmd(
        nc, in_maps, core_ids=list(range(n_cores))
    )
    return gather_output(res.results, B, T, C, n_cores)


# revision 18
# speedup vs baseline: 1.5156x; 1.0033x over previous
"""Causal self-attention on 8 Trainium2 NeuronCores (Bass/Tile).

Problem: nn_CausalSelfAttention (B=4, T=2048, C=1024, H=16 heads, fp32).

Sharding: tensor-parallel over heads for QKV projection + attention
(2 heads per core), per-batch AllGather of attention outputs (fp16,
transposed layout), then tensor-parallel over output columns for the
final projection (each core computes a 128-column slice of x@W_proj).
The AllGather/projection of batch b runs concurrently with the
attention of batch b+1.

Layouts (feature dim on partitions everywhere):
  xT      [C, B*T]        input, replicated to all cores
  Q^T,K^T [CH, B*T]       CH = channels per core (2 heads x 64)
  V       [B*T, CH]       natural layout (matmul lhsT for P@V), stored
                          tiled with an extra ones-column per head so the
                          PV matmul also produces softmax denominators
  S^T     [kr, q] tiles   scores transposed: kr on partitions -> exp'd
                          tiles feed P@V directly as the moving operand
  attn^T  [CH, T] fp16    per-core, per-batch -> AllGather -> [C, T]
  y^T     [OC, B*T]       per-core 128-column slice of the final output

Softmax: unnormalized exp (scores are O(1), no max subtraction needed);
causal mask = DVE add of a -1e5 upper-triangular [128,128] constant onto
the single diagonal-crossing strip of each clipped tile; denominator from
the V ones-column; division via reciprocal_approx_fast + PE broadcast.
Matmuls run as float32r (single-pass fp32); proj runs fp16 inputs.
"""

import numpy as np
from contextlib import ExitStack

P = 128
NQ = 512  # q/moving-operand tile width
MASKVAL = -1.0e5


def build_attention_nc(B, T, C, H, n_cores):
    import concourse.bass as bass  # noqa: F401
    import concourse.bacc as bacc
    import concourse.tile as tile
    import concourse.mybir as mybir

    f32 = mybir.dt.float32
    f32r = mybir.dt.float32r
    fp16 = mybir.dt.float16
    Identity = mybir.ActivationFunctionType.Identity
    Exp = mybir.ActivationFunctionType.Exp

    hs = C // H              # head size
    hpc = H // n_cores       # heads per core
    CH = hpc * hs            # qkv channels per core
    OC = C // n_cores        # output columns per core
    NT = B * T               # tokens
    KT_E = C // P            # contraction tiles over embedding dim
    NROW = NT // NQ          # token row-tiles
    TQ = T // NQ             # q tiles per batch
    TK = T // P              # kr tiles per batch
    TKALL = NT // P          # kr tiles over all batches
    DPB = NQ // P            # kr-tiles crossing one q-tile's diagonal
    WV = hpc * (hs + 1)      # V storage width per kr-tile (with ones cols)

    assert T % NQ == 0 and C % P == 0 and NT % NQ == 0
    assert CH <= P and H % n_cores == 0
    assert hs * hpc == CH and hpc in (1, 2)
    scale = 1.0 / float(np.sqrt(hs))

    nc = bacc.Bacc("TRN2", target_bir_lowering=False, debug=False,
                   num_devices=n_cores)

    xT = nc.dram_tensor("xT", [C, NT], f32r, kind="ExternalInput")
    wqkv = nc.dram_tensor("wqkv", [C, 3 * CH], f32r, kind="ExternalInput")
    bqkv = nc.dram_tensor("bqkv", [CH, 3], f32, kind="ExternalInput")
    wp = nc.dram_tensor("wp", [C, OC], fp16, kind="ExternalInput")
    bp = nc.dram_tensor("bp", [OC, 1], f32, kind="ExternalInput")
    yT = nc.dram_tensor("yT", [OC, NT], f32, kind="ExternalOutput")

    # constants baked into the NEFF
    ident_np = np.eye(P, dtype=np.float32)
    # mask[p, c] = MASKVAL where kr-offset p > q-offset c (strictly lower)
    mask_np = np.ascontiguousarray(np.where(
        np.arange(P)[:, None] > np.arange(P)[None, :], MASKVAL, 0.0
    ).astype(np.float32))
    ident_dram = nc.inline_tensor(ident_np, name="ident_const")
    mask_dram = nc.inline_tensor(mask_np, name="mask_const")
    ones_dram = nc.inline_tensor(np.ones((P, hs), dtype=np.float32),
                                 name="ones_const")
    vones_dram = nc.inline_tensor(np.ones((P, TKALL * hpc), dtype=np.float32),
                                  name="vones_const")

    with tile.TileContext(nc) as tc, ExitStack() as ctx:
        const = ctx.enter_context(tc.tile_pool(name="const", bufs=1))
        big = ctx.enter_context(tc.tile_pool(name="big", bufs=1))
        xin = ctx.enter_context(tc.tile_pool(name="xin", bufs=2))
        evac = ctx.enter_context(tc.tile_pool(name="evac", bufs=3))
        pexp = ctx.enter_context(tc.tile_pool(name="pexp", bufs=6))
        stp = ctx.enter_context(tc.tile_pool(name="stp", bufs=4, space="PSUM"))
        pvp = ctx.enter_context(tc.tile_pool(name="pvp", bufs=4, space="PSUM"))
        dram = ctx.enter_context(tc.tile_pool(name="dram", bufs=1, space="DRAM"))

        ident_t = const.tile([P, P], f32)
        mask_sb = const.tile([P, P], f32)
        ones_sb = const.tile([P, hs], f32r)
        bqkv_sb = const.tile([CH, 3], f32)
        bp_sb = const.tile([OC, 1], f32)
        w_sb = const.tile([P, KT_E * 3 * CH], f32r)
        wp_sb = const.tile([P, KT_E * OC], fp16)

        nc.sync.dma_start(ident_t[:], ident_dram[:])
        nc.sync.dma_start(mask_sb[:], mask_dram[:])
        nc.sync.dma_start(ones_sb[:], ones_dram[:].bitcast(f32r))
        nc.sync.dma_start(bqkv_sb[:], bqkv[:])
        nc.sync.dma_start(bp_sb[:], bp[:])
        nc.sync.dma_start(
            w_sb[:].rearrange("p (k m) -> p k m", k=KT_E),
            wqkv[:].rearrange("(k p) m -> p k m", p=P),
        )
        nc.sync.dma_start(
            wp_sb[:].rearrange("p (k m) -> p k m", k=KT_E),
            wp[:].rearrange("(k p) m -> p k m", p=P),
        )

        QT = big.tile([CH, NT], f32r)
        KTt = big.tile([CH, NT], f32r)
        Vn = big.tile([P, TKALL * WV], f32r)

        # ones columns of V (softmax denominator trick)
        ones_view = Vn[:].rearrange("p (v h d) -> p v h d", h=hpc, d=hs + 1)[
            :, :, :, hs:hs + 1
        ]
        nc.sync.dma_start(
            ones_view,
            vones_dram[:].rearrange("p (v h d) -> p v h d", h=hpc, d=1)
            .bitcast(f32r),
        )

        # ---- Phase A: QKV projection (outputs transposed) ----
        for n in range(NROW):
            ns = n * NQ
            xt = xin.tile([P, KT_E * NQ], f32r, tag="xcol")
            nc.sync.dma_start(
                xt[:].rearrange("p (k q) -> p k q", k=KT_E),
                xT[:, ns:ns + NQ].rearrange("(k p) q -> p k q", p=P),
            )
            for m in range(3):
                ps = stp.tile([P, NQ], f32, tag="st")
                for k in range(KT_E):
                    nc.tensor.matmul(
                        ps[0:CH, :],
                        lhsT=w_sb[:, k * 3 * CH + m * CH:k * 3 * CH + (m + 1) * CH],
                        rhs=xt[:, k * NQ:(k + 1) * NQ],
                        start=(k == 0),
                        stop=(k == KT_E - 1),
                    )
                if m == 0:
                    nc.scalar.activation(QT[:, ns:ns + NQ], ps[0:CH, :],
                                         Identity, bias=bqkv_sb[:, 0:1])
                elif m == 1:
                    nc.scalar.activation(KTt[:, ns:ns + NQ], ps[0:CH, :],
                                         Identity, bias=bqkv_sb[:, 1:2])
                else:
                    vt = evac.tile([CH, NQ], f32, tag="vtmp")
                    nc.scalar.activation(vt[:], ps[0:CH, :],
                                         Identity, bias=bqkv_sb[:, 2:3])
                    # transpose the DPB [CH, P] blocks -> [P, CH] into one bank
                    tp = stp.tile([P, NQ], f32, tag="st")
                    for j in range(DPB):
                        nc.tensor.transpose(
                            tp[:, j * CH:(j + 1) * CH],
                            vt[:, j * P:(j + 1) * P],
                            ident_t[0:CH, 0:CH],
                        )
                    # scatter into V storage (skipping the ones columns)
                    vi0 = n * DPB
                    dst = Vn[:, vi0 * WV:(vi0 + DPB) * WV].rearrange(
                        "p (v h d) -> p v h d", h=hpc, d=hs + 1
                    )[:, :, :, 0:hs]
                    src = tp[:, 0:DPB * CH].rearrange(
                        "p (v h d) -> p v h d", h=hpc, d=hs)
                    nc.vector.tensor_copy(dst, src)

        # ---- Phase B + C interleaved per batch ----
        cc_ins = [dram.tile([CH, T], fp16, name=f"ccin{b}") for b in range(B)]
        cc_outs = [dram.tile([n_cores * CH, T], fp16, addr_space="Shared",
                             name=f"ccout{b}") for b in range(B)]

        def attention(b):
            # normalization of q-tile qt-1 is deferred into qt's kt-loop so
            # the in-order PE never stalls behind the 3.3us DVE reciprocal
            # (which would re-throttle the HAM clock gate every tile).
            prev = None

            def issue_recips(state):
                qt0, pvs0, recs0 = state
                for hh in range(hpc):
                    with nc.allow_low_precision(reason="recip for bcast"):
                        nc.vector.reciprocal(recs0[hh][hs:hs + 1, :],
                                             pvs0[hh][hs:hs + 1, :])

            def issue_norm_rest(state):
                qt0, pvs0, recs0 = state
                for hh in range(hpc):
                    bc = stp.tile([P, NQ], f32, tag="st")
                    nc.tensor.matmul(
                        bc[0:hs, :],
                        lhsT=ones_sb[hs:hs + 1, :],
                        rhs=recs0[hh][hs:hs + 1, :],
                        start=True,
                        stop=True,
                    )
                    bcs = evac.tile([hs, NQ], f32, tag="bcs")
                    nc.vector.tensor_copy(bcs[:], bc[0:hs, :])
                    ao = evac.tile([hs, NQ], fp16, tag="ao")
                    nc.vector.tensor_mul(ao[:], pvs0[hh][0:hs, :], bcs[:])
                    nc.sync.dma_start(
                        cc_ins[b][hh * hs:(hh + 1) * hs,
                                  qt0 * NQ:(qt0 + 1) * NQ],
                        ao[:],
                    )

            for qt in range(TQ):
                qs = b * T + qt * NQ
                nkt = DPB * qt + DPB
                pvs = [pvp.tile([P, NQ], f32, tag="pv", name=f"pv{_h}")
                       for _h in range(hpc)]
                pes = {}

                def issue_st(kt, qt=qt, qs=qs, pes=pes):
                    ks = b * T + kt * P
                    diag = kt >= DPB * qt
                    j = kt - DPB * qt
                    c0 = j * P if diag else 0
                    for hh in range(hpc):
                        st = stp.tile([P, NQ], f32, tag="st", name=f"st{hh}")
                        nc.tensor.matmul(
                            st[:, c0:NQ],
                            lhsT=KTt[hh * hs:(hh + 1) * hs, ks:ks + P],
                            rhs=QT[hh * hs:(hh + 1) * hs, qs + c0:qs + NQ],
                            start=True,
                            stop=True,
                        )
                        if diag:
                            nc.vector.tensor_add(
                                st[:, c0:c0 + P], st[:, c0:c0 + P], mask_sb[:])
                        pe_t = pexp.tile([P, NQ], f32r, tag="pe",
                                         name=f"pe{hh}")
                        nc.scalar.activation(pe_t[:, c0:NQ], st[:, c0:NQ],
                                             Exp, scale=scale)
                        pes[(kt, hh)] = (pe_t, c0)

                def issue_pv(kt, nkt=nkt, pvs=pvs, pes=pes):
                    vi = b * TK + kt
                    for hh in range(hpc):
                        pe_t, c0 = pes.pop((kt, hh))
                        nc.tensor.matmul(
                            pvs[hh][0:hs + 1, c0:NQ],
                            lhsT=Vn[:, vi * WV + hh * (hs + 1):
                                    vi * WV + (hh + 1) * (hs + 1)],
                            rhs=pe_t[:, c0:NQ],
                            start=(kt == 0),
                            stop=(kt == nkt - 1),
                        )

                for kt in range(nkt):
                    issue_st(kt)
                    if kt == 3 and prev is not None:
                        issue_norm_rest(prev)
                    if kt >= 2:
                        issue_pv(kt - 2)
                for kt in range(max(0, nkt - 2), nkt):
                    issue_pv(kt)

                recs = [evac.tile([P, NQ], f32r, tag="rec", name=f"rec{_h}",
                                  bufs=2 * hpc)
                        for _h in range(hpc)]
                prev = (qt, pvs, recs)
                issue_recips(prev)

            issue_norm_rest(prev)

        def proj(b):
            for n in range(T // NQ):
                ns = n * NQ
                rt = xin.tile([P, KT_E * NQ], fp16, tag="xcol")
                nc.sync.dma_start(
                    rt[:].rearrange("p (k q) -> p k q", k=KT_E),
                    cc_outs[b][:, ns:ns + NQ].rearrange("(k p) q -> p k q",
                                                        p=P),
                )
                ps = stp.tile([P, NQ], f32, tag="st")
                for k in range(KT_E):
                    nc.tensor.matmul(
                        ps[0:OC, :],
                        lhsT=wp_sb[:, k * OC:(k + 1) * OC],
                        rhs=rt[:, k * NQ:(k + 1) * NQ],
                        start=(k == 0),
                        stop=(k == KT_E - 1),
                    )
                yo = evac.tile([OC, NQ], f32, tag="yo")
                nc.scalar.activation(yo[:], ps[0:OC, :], Identity,
                                     bias=bp_sb[:, 0:1])
                nc.sync.dma_start(yT[:, b * T + ns:b * T + ns + NQ], yo[:])

        for b in range(B):
            attention(b)
            nc.gpsimd.collective_compute(
                "AllGather",
                mybir.AluOpType.bypass,
                replica_groups=[list(range(n_cores))],
                ins=[cc_ins[b][:].opt()],
                outs=[cc_outs[b][:].opt()],
            )
            if b >= 1:
                proj(b - 1)
        proj(B - 1)

    nc.compile()
    return nc


def shard_inputs(x, W_qkv, b_qkv, W_proj, b_proj, H, n_cores):
    B, T, C = x.shape
    hs = C // H
    hpc = H // n_cores
    CH = hpc * hs
    OC = C // n_cores
    x2 = np.asarray(x, dtype=np.float32).reshape(B * T, C)
    xT = np.ascontiguousarray(x2.T)
    W_qkv = np.asarray(W_qkv, dtype=np.float32)
    b_qkv = np.asarray(b_qkv, dtype=np.float32)
    W_proj = np.asarray(W_proj, dtype=np.float32)
    b_proj = np.asarray(b_proj, dtype=np.float32)
    in_maps = []
    for i in range(n_cores):
        sl = slice(i * CH, (i + 1) * CH)
        wqkv_i = np.ascontiguousarray(np.concatenate(
            [W_qkv[:, sl], W_qkv[:, C:][:, sl], W_qkv[:, 2 * C:][:, sl]], axis=1))
        bqkv_i = np.ascontiguousarray(np.stack(
            [b_qkv[sl], b_qkv[C:][sl], b_qkv[2 * C:][sl]], axis=1))
        wp_i = np.ascontiguousarray(
            W_proj[:, i * OC:(i + 1) * OC].astype(np.float16))
        bp_i = np.ascontiguousarray(b_proj[i * OC:(i + 1) * OC].reshape(OC, 1))
        in_maps.append({"xT": xT, "wqkv": wqkv_i, "bqkv": bqkv_i,
                        "wp": wp_i, "bp": bp_i})
    return in_maps


def gather_output(results, B, T, C, n_cores):
    yT = np.concatenate([results[i]["yT"] for i in range(n_cores)], axis=0)
    return np.ascontiguousarray(yT.T).reshape(B, T, C).astype(np.float32)


_NC_CACHE = {}


def _get_nc(B, T, C, H, n_cores):
    key = (B, T, C, H, n_cores)
    if key not in _NC_CACHE:
        _NC_CACHE[key] = build_attention_nc(B, T, C, H, n_cores)
    return _NC_CACHE[key]


def kernel(x, W_qkv, b_qkv, W_proj, b_proj):
    from concourse import bass_utils

    B, T, C = 4, 2048, 1024
    H, n_cores = 16, 8
    assert x.shape == (B, T, C)
    nc = _get_nc(B, T, C, H, n_cores)
    in_maps = shard_inputs(x, W_qkv, b_qkv, W_proj, b_proj, H, n_cores)
    res = bass_utils.run_bass_kernel_spmd(
        nc, in_maps, core_ids=list(range(n_cores))
    )
    return gather_output(res.results, B, T, C, n_cores)
